# revision 1
# baseline (speedup 1.0000x reference)
"""CDGRL (gnn_message_passing) Trainium2 kernel — 8-core SPMD.

Row sharding, 512 rows/core. Each core builds the column-block A[:, r_c] of the
symmetric normalized adjacency in SBUF and uses it directly as lhsT for both
GCN propagations (A.T = A). Same-domain S blocks are structurally zero: the S
phase and both GCN t-loops branch on partition_id (tc.If) so each core only
computes its 16 inter-domain i-tiles; A's diagonal enters via four local
diag-tile matmuls whose lhsT comes from the core's own pre-AllGather block.
bf16 for large matmul streams (loss rel-err ~1e-7 vs f64 in emulation),
fp32-native matmul for the floor-sensitive centroid path.
"""

import numpy as np
import ml_dtypes

N = 4096
D = 4096
K = 21
NC = 8
R = 512
RT = 4            # 128-row tiles per core
DT = 32           # 128-chunks of D / of N
EPS = 1e-8
F1 = 2048
F2 = 1024
F3 = 512
F4 = 256


def _build():
    import concourse.bass as bass
    import concourse.mybir as mybir
    import concourse.tile as tile
    from concourse import bacc
    from concourse.masks import make_identity

    dt = mybir.dt
    AX = mybir.AxisListType.X
    OP = mybir.AluOpType
    ACT = mybir.ActivationFunctionType

    nc = bacc.Bacc("TRN2", target_bir_lowering=False, debug=False, num_devices=NC)

    xb = nc.dram_tensor("xb", [R, D], dt.float32, kind="ExternalInput")
    ohc_d = nc.dram_tensor("ohc", [R, K], dt.float32, kind="ExternalInput")
    ohdiv_d = nc.dram_tensor("ohdiv", [R, K], dt.float32, kind="ExternalInput")
    dmask_d = nc.dram_tensor("dmask", [1, DT], dt.float32, kind="ExternalInput")
    w1_d = nc.dram_tensor("w1b", [D, F1], dt.bfloat16, kind="ExternalInput")
    w2_d = nc.dram_tensor("w2b", [F1, F2], dt.bfloat16, kind="ExternalInput")
    fw1_d = nc.dram_tensor("fw1b", [F2, F3], dt.bfloat16, kind="ExternalInput")
    fw2_d = nc.dram_tensor("fw2b", [F3, F4], dt.bfloat16, kind="ExternalInput")
    fw3_d = nc.dram_tensor("fw3b", [F4, K], dt.bfloat16, kind="ExternalInput")
    b1t_d = nc.dram_tensor("b1t", [128, F1 // 128], dt.float32, kind="ExternalInput")
    b2t_d = nc.dram_tensor("b2t", [128, F2 // 128], dt.float32, kind="ExternalInput")
    fb1t_d = nc.dram_tensor("fb1t", [128, F3 // 128], dt.float32, kind="ExternalInput")
    fb2t_d = nc.dram_tensor("fb2t", [128, F4 // 128], dt.float32, kind="ExternalInput")
    fb3_d = nc.dram_tensor("fb3c", [K, 1], dt.float32, kind="ExternalInput")
    loss_d = nc.dram_tensor("loss", [1, 1], dt.float32, kind="ExternalOutput")

    def bcast(ap, n=128):
        return ap.partition_broadcast(n).rearrange("p one n -> p (one n)")

    with tile.TileContext(nc) as tc:
        with (
            tc.tile_pool(name="dram", bufs=1, space="DRAM") as dram,
            tc.tile_pool(name="pers", bufs=1) as pers,
            tc.tile_pool(name="pp_g", bufs=4, space="PSUM") as pp_g,
            tc.tile_pool(name="pp_s", bufs=2, space="PSUM") as pp_s,
            tc.tile_pool(name="pp_sm", bufs=2, space="PSUM") as pp_sm,
        ):
            # ---- collective DRAM buffers ----
            xnt_in = dram.tile([D * R], dt.bfloat16)
            xnt_all = dram.tile([NC, D, R], dt.bfloat16, addr_space="Shared")
            q_in = dram.tile([D * K], dt.float32)
            q_out = dram.tile([D * K], dt.float32)
            cm_in = dram.tile([K], dt.float32)
            cm_out = dram.tile([K], dt.float32)
            wv_in = dram.tile([R], dt.float32)
            wv_all = dram.tile([N], dt.float32, addr_space="Shared")
            deg_in = dram.tile([N], dt.float32)
            deg_out = dram.tile([N], dt.float32, addr_space="Shared")
            deg_rs = dram.tile([R], dt.float32)
            xw1_in = dram.tile([R * F1], dt.bfloat16)
            xw1_all = dram.tile([NC, R, F1], dt.bfloat16, addr_space="Shared")
            h2_in0 = dram.tile([R * 512], dt.bfloat16)
            h2_in1 = dram.tile([R * 512], dt.bfloat16)
            h2_all0 = dram.tile([NC, R, 512], dt.bfloat16, addr_space="Shared")
            h2_all1 = dram.tile([NC, R, 512], dt.bfloat16, addr_space="Shared")
            ls_in = dram.tile([1], dt.float32)
            ls_out = dram.tile([1], dt.float32, addr_space="Shared")

            GRP_ALL = [list(range(NC))]
            GRP_DOM = [[0, 1, 2, 3], [4, 5, 6, 7]]

            def cc(kind, op, i, o, groups):
                nc.gpsimd.collective_compute(
                    kind, op, replica_groups=groups, ins=[i.opt()], outs=[o.opt()]
                )

            # ---- persistent SBUF (~112 KB/partition) ----
            eye_bf = pers.tile([128, 128], dt.bfloat16)
            make_identity(nc, eye_bf[:])
            eye_f = pers.tile([128, 128], dt.float32)
            make_identity(nc, eye_f[:])
            ones_c = pers.tile([128, 1], dt.float32)
            nc.vector.memset(ones_c[:], 1.0)
            ohc = pers.tile([128, RT, K], dt.float32)
            nc.sync.dma_start(ohc[:], ohc_d.rearrange("(t p) k -> p t k", p=128))
            ohdiv = pers.tile([128, RT, K], dt.float32)
            nc.sync.dma_start(ohdiv[:], ohdiv_d.rearrange("(t p) k -> p t k", p=128))
            dmask = pers.tile([1, DT], dt.float32)
            nc.sync.dma_start(dmask[:], dmask_d[:])
            dmb = pers.tile([128, DT], dt.float32)
            nc.gpsimd.partition_broadcast(dmb[:], dmask[:])
            b1t = pers.tile([128, F1 // 128], dt.float32)
            nc.sync.dma_start(b1t[:], b1t_d[:])
            b2t = pers.tile([128, F2 // 128], dt.float32)
            nc.sync.dma_start(b2t[:], b2t_d[:])
            fb1t = pers.tile([128, F3 // 128], dt.float32)
            nc.sync.dma_start(fb1t[:], fb1t_d[:])
            fb2t = pers.tile([128, F4 // 128], dt.float32)
            nc.sync.dma_start(fb2t[:], fb2t_d[:])
            fb3 = pers.tile([K, 1], dt.float32)
            nc.sync.dma_start(fb3[:], fb3_d[:])

            xnT = pers.tile([128, DT * R], dt.bfloat16)
            A_sb = pers.tile([128, DT * R], dt.bfloat16)
            Adiag = pers.tile([128, RT * R], dt.bfloat16)
            XW1 = pers.tile([128, RT * F1], dt.bfloat16)
            h1T = pers.tile([128, (F1 // 128) * R], dt.bfloat16)
            H2s = pers.tile([128, RT * F2], dt.bfloat16)
            wj = pers.tile([1, R], dt.float32)
            wjb = pers.tile([128, R], dt.float32)
            wi = pers.tile([128, DT], dt.float32)
            dinv_i = pers.tile([128, DT], dt.float32)
            dinv_own = pers.tile([128, RT], dt.float32)
            dinvj = pers.tile([1, R], dt.float32)
            dinvjb = pers.tile([128, R], dt.float32)
            norm_r = pers.tile([128, RT], dt.float32)
            norm_b = pers.tile([128, RT], dt.float32)
            ninv_r = pers.tile([128, RT], dt.float32)
            ninvj = pers.tile([1, R], dt.float32)
            ninvjb = pers.tile([128, R], dt.float32)
            deg_sb = pers.tile([128, DT], dt.float32)
            simi = pers.tile([128, RT], dt.float32)

            # ============ P0: x load, Q partial, norms, xn, transpose, AGs ===
            with tc.tile_pool(name="p0", bufs=1) as p0:
                xrow = p0.tile([128, RT, D], dt.float32)
                nc.sync.dma_start(xrow[:], xb.rearrange("(t p) d -> p t d", p=128))

                # Q partial = x.T @ ohdiv (fp32-native, exact)
                q_sb = p0.tile([128, DT * K], dt.float32)
                for dtl in range(DT):
                    ps = pp_sm.tile([128, K], dt.float32, tag="sm", name=f"qp{dtl}")
                    for t in range(RT):
                        nc.tensor.matmul(
                            ps[:], xrow[:, t, 128 * dtl : 128 * (dtl + 1)],
                            ohdiv[:, t, :], start=(t == 0), stop=(t == RT - 1),
                        )
                    nc.vector.tensor_copy(q_sb[:, K * dtl : K * (dtl + 1)], ps[:])
                nc.sync.dma_start(
                    q_in[:].rearrange("(k p j) -> p k j", k=DT, p=128),
                    q_sb[:].rearrange("p (k j) -> p k j", k=DT),
                )
                cc("AllReduce", OP.add, q_in, q_out, GRP_DOM)

                # norms via ACT Square accumulate (two half-D passes)
                for t in range(RT):
                    sq = p0.tile([128, D // 2], dt.float32, tag="sq", bufs=2, name=f"sq{t}")
                    nc.scalar.activation(
                        sq[:], xrow[:, t, 0 : D // 2], ACT.Square,
                        accum_out=norm_r[:, t : t + 1],
                    )
                    sq2 = p0.tile([128, D // 2], dt.float32, tag="sq", bufs=2, name=f"sq2{t}")
                    nc.scalar.activation(
                        sq2[:], xrow[:, t, D // 2 : D], ACT.Square,
                        accum_out=norm_b[:, t : t + 1],
                    )
                nc.vector.tensor_tensor(norm_r[:], norm_r[:], norm_b[:], OP.add)
                nc.scalar.activation(norm_r[:], norm_r[:], ACT.Sqrt)
                nc.vector.tensor_scalar(ninv_r[:], norm_r[:], EPS, None, OP.max)
                nc.vector.reciprocal(ninv_r[:], ninv_r[:])
                for t in range(RT):
                    pw = pp_sm.tile([1, 128], dt.float32, tag="sm", name=f"nv{t}")
                    nc.tensor.transpose(pw[:], ninv_r[:, t : t + 1], eye_f[:])
                    nc.vector.tensor_copy(ninvj[:, 128 * t : 128 * (t + 1)], pw[:])
                nc.gpsimd.partition_broadcast(ninvjb[:], ninvj[:])

                # transpose raw x; normalize during PSUM evacuation (f32 mult
                # then bf16 round — bit-identical to scaling before transpose)
                for t in range(RT):
                    for k in range(DT):
                        ps = pp_sm.tile([128, 128], dt.float32, tag="sm", name=f"tp{t}_{k}")
                        nc.tensor.transpose(
                            ps[:], xrow[:, t, 128 * k : 128 * (k + 1)], eye_f[:]
                        )
                        nc.vector.tensor_tensor(
                            xnT[:, R * k + 128 * t : R * k + 128 * (t + 1)],
                            ps[:], ninvjb[:, 128 * t : 128 * (t + 1)], OP.mult,
                        )
                nc.sync.dma_start(
                    xnt_in[:].rearrange("(k p j) -> p k j", k=DT, p=128),
                    xnT[:].rearrange("p (k j) -> p k j", k=DT),
                )
                cc("AllGather", OP.bypass, xnt_in, xnt_all, GRP_ALL)

            # ============ centroid path: ct, Zn, simi, clsmax, w ============
            with tc.tile_pool(name="cen", bufs=1) as cen:
                q2 = cen.tile([128, DT * K], dt.float32)
                nc.sync.dma_start(
                    q2[:].rearrange("p (k j) -> p k j", k=DT),
                    q_out[:].rearrange("(k p j) -> p k j", k=DT, p=128),
                )
                cti = cen.tile([128, DT * K], dt.int32)
                nc.vector.tensor_copy(cti[:], q2[:])
                ctf = cen.tile([128, DT * K], dt.float32)
                nc.vector.tensor_copy(ctf[:], cti[:])
                ltq = cen.tile([128, DT * K], dt.float32)
                nc.vector.tensor_tensor(ltq[:], q2[:], ctf[:], OP.is_lt)
                ct = cen.tile([128, DT * K], dt.float32)
                nc.vector.tensor_tensor(ct[:], ctf[:], ltq[:], OP.subtract)
                ct_bf = cen.tile([128, DT * K], dt.bfloat16)
                nc.vector.tensor_copy(ct_bf[:], ct[:])

                ct2 = cen.tile([128, DT * K], dt.float32)
                nc.vector.tensor_tensor(ct2[:], ct[:], ct[:], OP.mult)
                cnp = cen.tile([1, DT * K], dt.float32)
                half = DT * K // 2
                for h in range(2):
                    ps = pp_sm.tile([1, half], dt.float32, tag="sm", name=f"cn{h}")
                    nc.tensor.matmul(
                        ps[:], ones_c[:], ct2[:, h * half : (h + 1) * half],
                        start=True, stop=True,
                    )
                    nc.vector.tensor_copy(cnp[:, h * half : (h + 1) * half], ps[:])
                cn = cen.tile([1, K], dt.float32)
                nc.vector.reduce_sum(
                    cn[:].rearrange("p (k one) -> p k one", one=1),
                    cnp[:].rearrange("p (k j) -> p j k", k=DT), axis=AX,
                )
                nc.scalar.activation(cn[:], cn[:], ACT.Sqrt)
                nc.vector.tensor_scalar(cn[:], cn[:], EPS, None, OP.max)
                cnb = cen.tile([128, K], dt.float32)
                nc.gpsimd.partition_broadcast(cnb[:], cn[:])

                msk = cen.tile([128, RT * K], dt.float32)
                for t in range(RT):
                    ps = pp_sm.tile([128, K], dt.float32, tag="sm", name=f"zn{t}")
                    for k in range(DT):
                        nc.tensor.matmul(
                            ps[:],
                            xnT[:, R * k + 128 * t : R * k + 128 * (t + 1)],
                            ct_bf[:, K * k : K * (k + 1)],
                            start=(k == 0), stop=(k == DT - 1),
                        )
                    sel = cen.tile([128, K], dt.float32, tag="sel", bufs=2, name=f"sel{t}")
                    nc.vector.tensor_tensor(sel[:], ps[:], ohc[:, t, :], OP.mult)
                    num = cen.tile([128, 1], dt.float32, tag="num", bufs=2, name=f"num{t}")
                    nc.vector.reduce_sum(num[:], sel[:], axis=AX, apply_absolute_value=True)
                    den = cen.tile([128, K], dt.float32, tag="den", bufs=2, name=f"den{t}")
                    nc.vector.tensor_tensor(den[:], ohc[:, t, :], cnb[:], OP.mult)
                    dens = cen.tile([128, 1], dt.float32, tag="dens", bufs=2, name=f"dens{t}")
                    nc.vector.reduce_sum(dens[:], den[:], axis=AX)
                    nc.vector.tensor_scalar(dens[:], dens[:], EPS, None, OP.max)
                    nc.vector.reciprocal(dens[:], dens[:])
                    nc.vector.tensor_tensor(simi[:, t : t + 1], num[:], dens[:], OP.mult)
                    nc.vector.tensor_scalar_mul(
                        msk[:, K * t : K * (t + 1)], ohc[:, t, :], simi[:, t : t + 1]
                    )
                m01 = cen.tile([128, K], dt.float32)
                nc.vector.tensor_tensor(m01[:], msk[:, 0:K], msk[:, K : 2 * K], OP.max)
                m23 = cen.tile([128, K], dt.float32)
                nc.vector.tensor_tensor(
                    m23[:], msk[:, 2 * K : 3 * K], msk[:, 3 * K : 4 * K], OP.max
                )
                mall = cen.tile([128, K], dt.float32)
                nc.vector.tensor_tensor(mall[:], m01[:], m23[:], OP.max)
                pst = pp_sm.tile([K, 128], dt.float32, tag="sm", name="cmt")
                nc.tensor.transpose(pst[:], mall[:], eye_f[:])
                cml = cen.tile([K, 1], dt.float32)
                nc.vector.reduce_max(cml[:], pst[:], axis=AX)
                nc.sync.dma_start(
                    cm_in[:].rearrange("(p one) -> p one", one=1), cml[:]
                )
                cc("AllReduce", OP.max, cm_in, cm_out, GRP_DOM)
                cmx = cen.tile([1, K], dt.float32)
                nc.sync.dma_start(
                    cmx[:], cm_out[:].rearrange("(one k) -> one k", one=1)
                )
                iszero = cen.tile([1, K], dt.float32)
                nc.vector.tensor_scalar(iszero[:], cmx[:], 0.0, None, OP.is_equal)
                nc.vector.tensor_tensor(cmx[:], cmx[:], iszero[:], OP.add)
                cmxb = cen.tile([128, K], dt.float32)
                nc.gpsimd.partition_broadcast(cmxb[:], cmx[:])

                wloc = cen.tile([128, RT], dt.float32)
                for t in range(RT):
                    mxs = cen.tile([128, K], dt.float32, tag="den", bufs=2, name=f"mxs{t}")
                    nc.vector.tensor_tensor(mxs[:], ohc[:, t, :], cmxb[:], OP.mult)
                    mxv = cen.tile([128, 1], dt.float32, tag="num", bufs=2, name=f"mxv{t}")
                    nc.vector.reduce_sum(mxv[:], mxs[:], axis=AX)
                    nc.vector.reciprocal(mxv[:], mxv[:])
                    nc.vector.tensor_tensor(
                        wloc[:, t : t + 1], simi[:, t : t + 1], mxv[:], OP.mult
                    )
                    nc.sync.dma_start(
                        wv_in[:].rearrange("(t p one) -> t p one", t=RT, one=1)[t],
                        wloc[:, t : t + 1],
                    )
                    pw = pp_sm.tile([1, 128], dt.float32, tag="sm", name=f"wt{t}")
                    nc.tensor.transpose(pw[:], wloc[:, t : t + 1], eye_f[:])
                    nc.vector.tensor_copy(wj[:, 128 * t : 128 * (t + 1)], pw[:])
                cc("AllGather", OP.bypass, wv_in, wv_all, GRP_ALL)
                nc.sync.dma_start(wi[:], wv_all[:].rearrange("(k p) -> p k", p=128))
                nc.gpsimd.partition_broadcast(wjb[:], wj[:])

            # ============ XnW1 = xn @ W1; XW1 = norm * XnW1; AG ============
            with tc.tile_pool(name="w1p", bufs=1) as w1p:
                for q in range(4):
                    w1q = w1p.tile([128, DT * 512], dt.bfloat16, tag="w1q", bufs=2, name=f"w1q{q}")
                    nc.sync.dma_start(
                        w1q[:].rearrange("p (k j) -> p k j", k=DT),
                        w1_d.rearrange("(k p) f -> p k f", p=128)[:, :, 512 * q : 512 * (q + 1)],
                    )
                    for t in range(RT):
                        ps = pp_g.tile([128, 512], dt.float32, tag="gc", name=f"xw_{q}_{t}")
                        for k in range(DT):
                            nc.tensor.matmul(
                                ps[:],
                                xnT[:, R * k + 128 * t : R * k + 128 * (t + 1)],
                                w1q[:, 512 * k : 512 * (k + 1)],
                                start=(k == 0), stop=(k == DT - 1),
                            )
                        nc.scalar.activation(
                            XW1[:, F1 * t + 512 * q : F1 * t + 512 * (q + 1)],
                            ps[:], ACT.Identity, scale=norm_r[:, t : t + 1],
                        )
                for t in range(RT):
                    nc.sync.dma_start(
                        xw1_in[:].rearrange("(t p f) -> t p f", t=RT, p=128)[t],
                        XW1[:, F1 * t : F1 * (t + 1)],
                    )
                cc("AllGather", OP.bypass, xw1_in, xw1_all, GRP_ALL)

            # ============ S phase (If-split: each core computes only its 16
            # inter-domain i-tiles; same-domain A/deg halves are zeroed) ======
            with tc.tile_pool(name="sph", bufs=1) as sph:
                def s_half(tiles, arm):
                    for t in tiles:
                        cprime, li = t // RT, t % RT
                        sps = pp_s.tile([128, R], dt.float32, tag="sp", name=f"sp{arm}_{t}")
                        for kg in range(8):
                            lt = sph.tile([128, 4, 128], dt.bfloat16, tag="lt", bufs=4, name=f"lt{arm}_{t}_{kg}")
                            nc.sync.dma_start(
                                lt[:],
                                xnt_all[cprime, 512 * kg : 512 * (kg + 1),
                                        128 * li : 128 * (li + 1)]
                                .rearrange("(kk p) j -> p kk j", p=128),
                            )
                            for kk in range(4):
                                k = 4 * kg + kk
                                nc.tensor.matmul(
                                    sps[:], lt[:, kk, :], xnT[:, R * k : R * (k + 1)],
                                    start=(k == 0), stop=(k == DT - 1),
                                )
                        sabs = sph.tile([128, R], dt.float32, tag="sabs", bufs=2, name=f"sa{arm}_{t}")
                        nc.scalar.activation(sabs[:], sps[:], ACT.Abs)
                        wd = sph.tile([128, R], dt.float32, tag="wd", bufs=2, name=f"wd{arm}_{t}")
                        nc.vector.tensor_scalar(
                            wd[:], wjb[:], wi[:, t : t + 1], None, OP.subtract
                        )
                        nc.scalar.activation(wd[:], wd[:], ACT.Abs)
                        u = sph.tile([128, R], dt.float32, tag="u", bufs=2, name=f"u{arm}_{t}")
                        nc.vector.tensor_tensor(u[:], wd[:], sabs[:], OP.mult)
                        nc.vector.tensor_tensor(u[:], sabs[:], u[:], OP.subtract)
                        nc.vector.reduce_sum(deg_sb[:, t : t + 1], u[:], axis=AX)
                        nc.vector.tensor_copy(A_sb[:, R * t : R * (t + 1)], u[:])

                pid = nc.partition_id()
                with tc.If(pid < 4) as cmp:
                    nc.vector.memset(deg_sb[:, 0:16], 0.0)
                    s_half(range(16, 32), 0)
                with cmp.Else():
                    nc.vector.memset(deg_sb[:, 16:32], 0.0)
                    s_half(range(0, 16), 1)
                nc.sync.dma_start(
                    deg_in[:].rearrange("(k p) -> p k", p=128), deg_sb[:]
                )
                cc("AllReduce", OP.add, deg_in, deg_out, GRP_ALL)
                cc("ReduceScatter", OP.add, deg_in, deg_rs, GRP_ALL)

                nc.sync.dma_start(
                    dinv_i[:], deg_out[:].rearrange("(k p) -> p k", p=128)
                )
                nc.vector.tensor_scalar_add(dinv_i[:], dinv_i[:], 1.0)
                nc.vector.reciprocal(dinv_i[:], dinv_i[:])
                nc.scalar.activation(dinv_i[:], dinv_i[:], ACT.Sqrt)
                nc.sync.dma_start(
                    dinv_own[:], deg_rs[:].rearrange("(t p) -> p t", p=128)
                )
                nc.vector.tensor_scalar_add(dinv_own[:], dinv_own[:], 1.0)
                nc.vector.reciprocal(dinv_own[:], dinv_own[:])
                nc.scalar.activation(dinv_own[:], dinv_own[:], ACT.Sqrt)
                for t in range(RT):
                    pw = pp_sm.tile([1, 128], dt.float32, tag="sm", name=f"dj{t}")
                    nc.tensor.transpose(pw[:], dinv_own[:, t : t + 1], eye_f[:])
                    nc.vector.tensor_copy(dinvj[:, 128 * t : 128 * (t + 1)], pw[:])
                nc.gpsimd.partition_broadcast(dinvjb[:], dinvj[:])
                def a_scale(tiles):
                    for t in tiles:
                        nc.vector.tensor_scalar_mul(
                            A_sb[:, R * t : R * (t + 1)],
                            A_sb[:, R * t : R * (t + 1)], dinv_i[:, t : t + 1],
                        )
                        nc.vector.tensor_tensor(
                            A_sb[:, R * t : R * (t + 1)],
                            A_sb[:, R * t : R * (t + 1)], dinvjb[:], OP.mult,
                        )
                with tc.If(pid < 4) as cmpa:
                    a_scale(range(16, 32))
                with cmpa.Else():
                    a_scale(range(0, 16))
                d2 = sph.tile([128, RT], dt.float32)
                nc.vector.tensor_tensor(d2[:], dinv_own[:], dinv_own[:], OP.mult)
                nc.vector.memset(Adiag[:], 0.0)
                for s in range(RT):
                    nc.vector.tensor_scalar_mul(
                        Adiag[:, R * s + 128 * s : R * s + 128 * (s + 1)],
                        eye_bf[:], d2[:, s : s + 1],
                    )

            # ============ GCN layer 1 + H2 + GCN layer 2 ============
            with tc.tile_pool(name="gcn", bufs=1) as gcn:
                w2qs = []
                for q in range(2):
                    w2q = gcn.tile([128, (F1 // 128) * 512], dt.bfloat16, tag="w2q", bufs=2, name=f"w2qp{q}")
                    nc.sync.dma_start(
                        w2q[:].rearrange("p (k j) -> p k j", k=F1 // 128),
                        w2_d.rearrange("(k p) f -> p k f", p=128)[:, :, 512 * q : 512 * (q + 1)],
                    )
                    w2qs.append(w2q)
                pidg = nc.partition_id()
                for fg in range(4):
                    pss = [
                        pp_g.tile([128, R], dt.float32, tag="gc", name=f"g1_{fg}_{ff}")
                        for ff in range(4)
                    ]
                    def g1_arm(tiles, arm, fg=fg, pss=pss):
                        first = tiles[0]
                        for t in tiles:
                            cprime, li = t // RT, t % RT
                            ld = gcn.tile([128, 512], dt.bfloat16, tag="ld", bufs=4, name=f"ld{fg}_{t}_{arm}")
                            nc.sync.dma_start(
                                ld[:],
                                xw1_all[cprime, 128 * li : 128 * (li + 1),
                                        512 * fg : 512 * (fg + 1)],
                            )
                            for ff in range(4):
                                nc.tensor.matmul(
                                    pss[ff][:], ld[:, 128 * ff : 128 * (ff + 1)],
                                    A_sb[:, R * t : R * (t + 1)],
                                    start=(t == first), stop=False,
                                )
                    with tc.If(pidg < 4) as cmpg:
                        g1_arm(list(range(16, 32)), 0)
                    with cmpg.Else():
                        g1_arm(list(range(0, 16)), 1)
                    for ff in range(4):
                        f = 4 * fg + ff
                        for s in range(RT):
                            nc.tensor.matmul(
                                pss[ff][:],
                                XW1[:, F1 * s + 128 * f : F1 * s + 128 * (f + 1)],
                                Adiag[:, R * s : R * (s + 1)],
                                start=False, stop=(s == RT - 1),
                            )
                        nc.scalar.activation(
                            h1T[:, R * f : R * (f + 1)], pss[ff][:], ACT.Relu,
                            bias=b1t[:, f : f + 1],
                        )
                # H2 = h1 @ W2 (no bias yet), AG
                for q in range(2):
                    w2q = w2qs[q]
                    for t in range(RT):
                        ps = pp_g.tile([128, 512], dt.float32, tag="gc", name=f"h2_{q}_{t}")
                        for k in range(F1 // 128):
                            nc.tensor.matmul(
                                ps[:],
                                h1T[:, R * k + 128 * t : R * k + 128 * (t + 1)],
                                w2q[:, 512 * k : 512 * (k + 1)],
                                start=(k == 0), stop=(k == F1 // 128 - 1),
                            )
                        nc.vector.tensor_copy(
                            H2s[:, F2 * t + 512 * q : F2 * t + 512 * (q + 1)], ps[:]
                        )
                    h2_in_q = h2_in0 if q == 0 else h2_in1
                    for t in range(RT):
                        nc.sync.dma_start(
                            h2_in_q[:].rearrange("(t p f) -> t p f", t=RT, p=128)[t],
                            H2s[:, F2 * t + 512 * q : F2 * t + 512 * (q + 1)],
                        )
                    cc("AllGather", OP.bypass, h2_in_q,
                       h2_all0 if q == 0 else h2_all1, GRP_ALL)

                with tc.tile_pool(name="cls", bufs=1) as cls:
                    h2T = cls.tile([128, (F2 // 128) * R], dt.bfloat16)
                    for fg in range(2):
                        pss = [
                            pp_g.tile([128, R], dt.float32, tag="gc", name=f"g2_{fg}_{ff}")
                            for ff in range(4)
                        ]
                        def g2_arm(tiles, arm, fg=fg, pss=pss):
                            first = tiles[0]
                            for t in tiles:
                                cprime, li = t // RT, t % RT
                                ld = gcn.tile([128, 512], dt.bfloat16, tag="ld", bufs=4, name=f"l2{fg}_{t}_{arm}")
                                h2src = h2_all0 if fg == 0 else h2_all1
                                nc.sync.dma_start(
                                    ld[:],
                                    h2src[cprime, 128 * li : 128 * (li + 1), :],
                                )
                                for ff in range(4):
                                    nc.tensor.matmul(
                                        pss[ff][:], ld[:, 128 * ff : 128 * (ff + 1)],
                                        A_sb[:, R * t : R * (t + 1)],
                                        start=(t == first), stop=False,
                                    )
                        with tc.If(pidg < 4) as cmp2:
                            g2_arm(list(range(16, 32)), 0)
                        with cmp2.Else():
                            g2_arm(list(range(0, 16)), 1)
                        for ff in range(4):
                            f = 4 * fg + ff
                            for s in range(RT):
                                nc.tensor.matmul(
                                    pss[ff][:],
                                    H2s[:, F2 * s + 128 * f : F2 * s + 128 * (f + 1)],
                                    Adiag[:, R * s : R * (s + 1)],
                                    start=False, stop=(s == RT - 1),
                                )
                            nc.scalar.activation(
                                h2T[:, R * f : R * (f + 1)], pss[ff][:], ACT.Identity,
                                bias=b2t[:, f : f + 1],
                            )

                    # classifier
                    fw1s = cls.tile([128, (F2 // 128) * F3], dt.bfloat16)
                    nc.sync.dma_start(
                        fw1s[:].rearrange("p (k j) -> p k j", k=F2 // 128),
                        fw1_d.rearrange("(k p) f -> p k f", p=128),
                    )
                    fw2s = cls.tile([128, (F3 // 128) * F4], dt.bfloat16)
                    nc.sync.dma_start(
                        fw2s[:].rearrange("p (k j) -> p k j", k=F3 // 128),
                        fw2_d.rearrange("(k p) f -> p k f", p=128),
                    )
                    fw3s = cls.tile([128, (F4 // 128) * K], dt.bfloat16)
                    nc.sync.dma_start(
                        fw3s[:].rearrange("p (k j) -> p k j", k=F4 // 128),
                        fw3_d.rearrange("(k p) f -> p k f", p=128),
                    )
                    h3T = cls.tile([128, (F3 // 128) * R], dt.bfloat16)
                    for f in range(F3 // 128):
                        ps = pp_g.tile([128, R], dt.float32, tag="gc", name=f"c1_{f}")
                        for k in range(F2 // 128):
                            nc.tensor.matmul(
                                ps[:],
                                fw1s[:, F3 * k + 128 * f : F3 * k + 128 * (f + 1)],
                                h2T[:, R * k : R * (k + 1)],
                                start=(k == 0), stop=(k == F2 // 128 - 1),
                            )
                        nc.scalar.activation(
                            h3T[:, R * f : R * (f + 1)], ps[:], ACT.Relu,
                            bias=fb1t[:, f : f + 1],
                        )
                    h4T = cls.tile([128, (F4 // 128) * R], dt.bfloat16)
                    for f in range(F4 // 128):
                        ps = pp_g.tile([128, R], dt.float32, tag="gc", name=f"c2_{f}")
                        for k in range(F3 // 128):
                            nc.tensor.matmul(
                                ps[:],
                                fw2s[:, F4 * k + 128 * f : F4 * k + 128 * (f + 1)],
                                h3T[:, R * k : R * (k + 1)],
                                start=(k == 0), stop=(k == F3 // 128 - 1),
                            )
                        nc.scalar.activation(
                            h4T[:, R * f : R * (f + 1)], ps[:], ACT.Relu,
                            bias=fb2t[:, f : f + 1],
                        )
                    pl = pp_sm.tile([K, R], dt.float32, tag="sm", name="lgp")
                    for k in range(F4 // 128):
                        nc.tensor.matmul(
                            pl[:], fw3s[:, K * k : K * (k + 1)],
                            h4T[:, R * k : R * (k + 1)],
                            start=(k == 0), stop=(k == F4 // 128 - 1),
                        )
                    lgt = cls.tile([K, R], dt.float32)
                    nc.scalar.activation(lgt[:], pl[:], ACT.Identity, bias=fb3[:])

                    # log-softmax + NLL + partial sum
                    pacc = pp_sm.tile([1, 1], dt.float32, tag="sm", name="lacc")
                    for t in range(RT):
                        pt = pp_s.tile([128, K], dt.float32, tag="sp", name=f"lgt{t}")
                        nc.tensor.transpose(
                            pt[:], lgt[:, 128 * t : 128 * (t + 1)],
                            eye_f[0:K, 0:K],
                        )
                        lgr = cls.tile([128, K], dt.float32, tag="lgr", bufs=2, name=f"lgr{t}")
                        nc.vector.tensor_copy(lgr[:], pt[:])
                        nmax = cls.tile([128, 1], dt.float32, tag="nmx", bufs=2, name=f"nmx{t}")
                        nc.vector.reduce_max(nmax[:], lgr[:], axis=AX, negate=True)
                        ex = cls.tile([128, K], dt.float32, tag="ex", bufs=2, name=f"ex{t}")
                        sumex = cls.tile([128, 1], dt.float32, tag="sx", bufs=2, name=f"sx{t}")
                        nc.scalar.activation(
                            ex[:], lgr[:], ACT.Exp, bias=nmax[:], accum_out=sumex[:]
                        )
                        lse = cls.tile([128, 1], dt.float32, tag="lse", bufs=2, name=f"lse{t}")
                        nc.scalar.activation(lse[:], sumex[:], ACT.Ln)
                        selm = cls.tile([128, K], dt.float32, tag="selm", bufs=2, name=f"selm{t}")
                        nc.vector.tensor_tensor(selm[:], lgr[:], ohc[:, t, :], OP.mult)
                        selv = cls.tile([128, 1], dt.float32, tag="selv", bufs=2, name=f"selv{t}")
                        nc.vector.reduce_sum(selv[:], selm[:], axis=AX)
                        nll = cls.tile([128, 1], dt.float32, tag="nll", bufs=2, name=f"nll{t}")
                        nc.vector.tensor_tensor(nll[:], lse[:], nmax[:], OP.subtract)
                        nc.vector.tensor_tensor(nll[:], nll[:], selv[:], OP.subtract)
                        nc.tensor.matmul(
                            pacc[:], ones_c[:], nll[:],
                            start=(t == 0), stop=(t == RT - 1),
                        )
                    lsum = cls.tile([1, 1], dt.float32)
                    nc.vector.tensor_copy(lsum[:], pacc[:])
                    nc.sync.dma_start(
                        ls_in[:].rearrange("(p one) -> p one", one=1), lsum[:]
                    )
                    cc("AllReduce", OP.add, ls_in, ls_out, GRP_ALL)
                    lf = cls.tile([1, 1], dt.float32)
                    nc.sync.dma_start(
                        lf[:], ls_out[:].rearrange("(one k) -> one k", one=1)
                    )
                    nc.vector.tensor_scalar_mul(lf[:], lf[:], 1.0 / N)
                    nc.sync.dma_start(loss_d[:], lf[:])

    nc.finalize()
    return nc


_NC_CACHE = None


def kernel(x1, x2, label1, label2, W1, b1, W2, b2,
           fw1, fb1, fw2, fb2, fw3, fb3):
    global _NC_CACHE
    from concourse.bass_utils import run_bass_kernel_spmd

    x = np.concatenate([np.asarray(x1, np.float32), np.asarray(x2, np.float32)], 0)
    label = np.concatenate([np.asarray(label1), np.asarray(label2)]).astype(np.int64)

    oh = np.zeros((N, K), np.float32)
    oh[np.arange(N), label] = 1.0
    su1 = np.maximum(oh[:2048].sum(0), 1.0)
    su2 = np.maximum(oh[2048:].sum(0), 1.0)
    ohdiv = np.concatenate([oh[:2048] / su1, oh[2048:] / su2], 0).astype(np.float32)

    bf = ml_dtypes.bfloat16
    w1b = np.asarray(W1, np.float32).astype(bf)
    w2b = np.asarray(W2, np.float32).astype(bf)
    fw1b = np.asarray(fw1, np.float32).astype(bf)
    fw2b = np.asarray(fw2, np.float32).astype(bf)
    fw3b = np.asarray(fw3, np.float32).astype(bf)
    b1t = np.ascontiguousarray(np.asarray(b1, np.float32).reshape(F1 // 128, 128).T)
    b2t = np.ascontiguousarray(np.asarray(b2, np.float32).reshape(F2 // 128, 128).T)
    fb1t = np.ascontiguousarray(np.asarray(fb1, np.float32).reshape(F3 // 128, 128).T)
    fb2t = np.ascontiguousarray(np.asarray(fb2, np.float32).reshape(F4 // 128, 128).T)
    fb3c = np.asarray(fb3, np.float32).reshape(K, 1)

    if _NC_CACHE is None:
        _NC_CACHE = _build()
    nc = _NC_CACHE

    in_maps = []
    for c in range(NC):
        rows = slice(R * c, R * (c + 1))
        dom_c = c >= 4
        dmask = np.array(
            [[0.0 if (t >= 16) == dom_c else 1.0 for t in range(DT)]], np.float32
        )
        in_maps.append({
            "xb": np.ascontiguousarray(x[rows]),
            "ohc": np.ascontiguousarray(oh[rows]),
            "ohdiv": np.ascontiguousarray(ohdiv[rows]),
            "dmask": dmask,
            "w1b": w1b, "w2b": w2b, "fw1b": fw1b, "fw2b": fw2b, "fw3b": fw3b,
            "b1t": b1t, "b2t": b2t, "fb1t": fb1t, "fb2t": fb2t, "fb3c": fb3c,
        })

    res = run_bass_kernel_spmd(nc, in_maps, list(range(NC)))
    return np.asarray(res.results[0]["loss"], np.float32).reshape(())

